# revision 7
# baseline (speedup 1.0000x reference)
"""Trainium2 Bass kernel for masked dot-product attention variant:

    out[b,p,l,m] = (sum_d Q[b,p,l,d] K[b,p,m,d]) / sqrt(D) * mask[b,p] * V[b,p,l,m]

Sharding: data-parallel over batch dim B=16 -> 2 batches per core on 8 cores.
Per core: 128 independent (b,p) pairs, each a 256x128 @ 128x256 gemm plus an
elementwise multiply with V (mask/sqrt(D) is folded into Q on the host).

All bulk I/O moves in bf16 (harness gate is rel_err < 2e-2; bf16 rounding
contributes ~5e-3), halving HBM traffic vs fp32. Layouts are chosen so every
steady-state DMA descriptor covers a 16KB contiguous run (measured ~410ns per
16KB descriptor vs 306ns per 8KB -> ~40GB/s per DMA engine):

  qt/kt[lg*128 + d, :] = 32 pairs packed per 128-row block (partition d)
  v/out[cg*128 + p, :] = 16 pairs per block; partition p holds rows l=2p,2p+1
                         (column-interleaved scores match this layout)

Per pair the PE computes scores[l_chunk, m] = qT[:, l_chunk].T @ kT in fp32
PSUM; one DVE scalar_tensor_tensor per FOUR pairs does out = scores * V with
bf16 output (batched to amortize DVE per-op overhead; ~74us DVE busy).
DMA rings: q/k loads on SP, v loads on GpSimd (SWDGE), stores on ACT - each
ring is homogeneous so loads never head-of-line block behind stores.
"""

import numpy as np

B, P, L, D = 16, 64, 256, 128
NCORES = 8
BPC = B // NCORES          # batches per core = 2
PAIRS = BPC * P            # (b,p) pairs per core = 128
CG = 16                    # pairs per compute/store group (v/out row block)
LG = 32                    # pairs per q/k load group (q/k row block)

ISQRT_D = 1.0 / np.sqrt(D)


def build_bass(pairs=PAIRS, qk_bufs=2, io_bufs=3, sc_bufs=2, dve_pairs=4,
               head_split=2, tail_osplit=4, store_split=1, v_ring="gpsimd"):
    import concourse.bacc as bacc
    import concourse.mybir as mybir
    import concourse.tile as tile
    from concourse.bass import ds, ts

    f32 = mybir.dt.float32
    bf16 = mybir.dt.bfloat16
    lgroups = pairs // LG      # 4
    cgroups = pairs // CG      # 8
    qw = LG * 256              # q/k row width (elements) = 8192
    vw = CG * 512              # v/out row width = 8192
    nc = bacc.Bacc("TRN2")

    # qt row (lg*128+d): [j32, c, p'] for the 32 pairs of load-group lg; l=2p'+c
    qt = nc.dram_tensor("qt", [lgroups * 128, qw], bf16, kind="ExternalInput")
    # kt row (lg*128+d): [j32, m]
    kt = nc.dram_tensor("kt", [lgroups * 128, qw], bf16, kind="ExternalInput")
    # v row (cg*128+p): [j16, c, x] = V[pair j, l=2p+c, x]
    v = nc.dram_tensor("v", [cgroups * 128, vw], bf16, kind="ExternalInput")
    out = nc.dram_tensor("out", [cgroups * 128, vw], bf16, kind="ExternalOutput")

    mult = mybir.AluOpType.mult
    vload = {"gpsimd": nc.gpsimd, "sync": nc.sync, "vector": nc.vector}[v_ring]
    dw = dve_pairs * 512       # elements per DVE op

    with tile.TileContext(nc) as tc:
        with (
            tc.tile_pool(name="qk", bufs=qk_bufs) as qk,
            tc.tile_pool(name="io", bufs=io_bufs) as io,
            tc.tile_pool(name="pss", bufs=sc_bufs, space="PSUM") as pss,
        ):
            for lg in range(lgroups):
                r0 = lg * 128
                qn = qk.tile([128, qw], bf16, tag="qn")
                kn = qk.tile([128, qw], bf16, tag="kn")
                nsplit = head_split if lg == 0 else 1
                qh = qw // nsplit
                for s in range(nsplit):
                    nc.sync.dma_start(
                        out=qn[:, ds(s * qh, qh)],
                        in_=qt[r0 : r0 + 128, s * qh : (s + 1) * qh],
                    )
                    nc.sync.dma_start(
                        out=kn[:, ds(s * qh, qh)],
                        in_=kt[r0 : r0 + 128, s * qh : (s + 1) * qh],
                    )

                for h in range(2):
                    cg = 2 * lg + h
                    c0 = cg * 128
                    vn = io.tile([128, vw], bf16, tag="vn")
                    osb = io.tile([128, vw], bf16, tag="osb")
                    vsplit = head_split if cg == 0 else 1
                    vh = vw // vsplit
                    for s in range(vsplit):
                        vload.dma_start(
                            out=vn[:, ds(s * vh, vh)],
                            in_=v[c0 : c0 + 128, s * vh : (s + 1) * vh],
                        )

                    osplit = tail_osplit if cg == cgroups - 1 else store_split
                    nops = CG // dve_pairs
                    for u in range(nops):
                        sc = pss.tile([128, dve_pairs * 512], f32, tag="sc")
                        for q in range(dve_pairs):
                            j = dve_pairs * u + q          # pair within CG
                            jcol = (h * CG + j) * 256      # column in qn/kn
                            for r in range(2):
                                nc.tensor.matmul(
                                    sc[:, ds(q * 512 + r * 256, 256)],
                                    lhsT=qn[:, ds(jcol + r * 128, 128)],
                                    rhs=kn[:, ds(jcol, 256)],
                                    start=True,
                                    stop=True,
                                )
                        nc.vector.scalar_tensor_tensor(
                            out=osb[:, ds(u * dw, dw)],
                            in0=sc[:, ds(0, dw)],
                            scalar=1.0,
                            in1=vn[:, ds(u * dw, dw)],
                            op0=mult,
                            op1=mult,
                        )
                        per = nops // osplit
                        if (u + 1) % per == 0:
                            s0 = (u + 1 - per) * dw
                            nc.scalar.dma_start(
                                out=out[c0 : c0 + 128, s0 : s0 + per * dw],
                                in_=osb[:, ds(s0, per * dw)],
                            )
    nc.finalize()
    return nc


def make_in_maps(queries, keys, values, mask, ncores=NCORES):
    import ml_dtypes

    bf16 = ml_dtypes.bfloat16
    lgroups = PAIRS // LG
    cgroups = PAIRS // CG
    queries = np.asarray(queries, dtype=np.float32)
    keys = np.asarray(keys, dtype=np.float32)
    values = np.asarray(values, dtype=np.float32)
    mask = np.asarray(mask, dtype=np.float32)
    in_maps = []
    for c in range(ncores):
        bs = slice(c * BPC, (c + 1) * BPC)
        mrow = mask[bs].reshape(PAIRS) * ISQRT_D
        qs = queries[bs].reshape(PAIRS, L, D) * mrow[:, None, None]
        ks = keys[bs].reshape(PAIRS, L, D)
        # qt: [lg, j, p', c, d] -> [lg, d, j, c, p'] ; l = 2p'+c
        qtp = (
            qs.reshape(lgroups, LG, 128, 2, D)
            .transpose(0, 4, 1, 3, 2)
            .reshape(lgroups * 128, LG * 256)
        )
        # kt: [lg, j, m, d] -> [lg, d, j, m]
        ktp = (
            ks.reshape(lgroups, LG, 256, D)
            .transpose(0, 3, 1, 2)
            .reshape(lgroups * 128, LG * 256)
        )
        # v: [cg, j, p, c, x] -> [cg, p, j, c, x] ; row l = 2p+c
        vp = (
            values[bs]
            .reshape(cgroups, CG, 128, 2, 256)
            .transpose(0, 2, 1, 3, 4)
            .reshape(cgroups * 128, CG * 512)
        )
        in_maps.append(
            {
                "qt": np.ascontiguousarray(qtp).astype(bf16),
                "kt": np.ascontiguousarray(ktp).astype(bf16),
                "v": np.ascontiguousarray(vp).astype(bf16),
            }
        )
    return in_maps


def unpack_out(arr):
    """[cgroups*128, CG*512] device layout -> [BPC, P, L, L] fp32."""
    cgroups = PAIRS // CG
    a = arr.astype(np.float32).reshape(cgroups, 128, CG, 2, 256)
    a = a.transpose(0, 2, 1, 3, 4).reshape(BPC, P, L, L)
    return a


def run(queries, keys, values, mask, trace=False, **build_kwargs):
    """Build, compile and run on 8 cores; returns (full_output, BassKernelResults)."""
    from concourse.bass_utils import run_bass_kernel_spmd

    nc = build_bass(**build_kwargs)
    in_maps = make_in_maps(queries, keys, values, mask)
    res = run_bass_kernel_spmd(
        nc, in_maps, core_ids=list(range(NCORES)), trace=trace
    )
    outs = [unpack_out(r["out"]) for r in res.results]
    return np.concatenate(outs, axis=0), res


def kernel(queries, keys, values, mask):
    out, _ = run(queries, keys, values, mask, trace=False)
    return out


# revision 8
# speedup vs baseline: 1.1616x; 1.1616x over previous
"""Trainium2 Bass kernel for masked dot-product attention variant:

    out[b,p,l,m] = (sum_d Q[b,p,l,d] K[b,p,m,d]) / sqrt(D) * mask[b,p] * V[b,p,l,m]

Sharding: data-parallel over batch dim B=16 -> 2 batches per core on 8 cores.
Per core: 128 independent (b,p) pairs, each a 256x128 @ 128x256 gemm plus an
elementwise multiply with V (mask/sqrt(D) is folded into Q on the host).

All bulk I/O moves in bf16 (harness gate is rel_err < 2e-2; bf16 rounding
contributes ~5e-3), halving HBM traffic vs fp32. Layouts give 8KB (q/k) and
16KB (v/out) contiguous runs per DMA descriptor; with all 8 cores pulling,
the chip HBM wall (~2.6-2.8TB/s) caps each core around 330-400GB/s, so the
kernel is paced by its ~126us of DMA-engine busy time per core.

  qt/kt[g*128 + d, :] = 16 pairs packed per 128-row block (partition d)
  v/out[g*128 + p, :] = partition p holds rows l=2p, 2p+1 of each pair
                        (column-interleaved scores match this layout)

Per pair the PE computes scores[l_chunk, m] = qT[:, l_chunk].T @ kT in fp32
PSUM; one DVE scalar_tensor_tensor per FOUR pairs does out = scores * V with
bf16 output (batched to amortize DVE per-op overhead; ~72us DVE busy).
DMA rings: q/k loads on SP, v loads on GpSimd (SWDGE), stores on ACT - each
ring homogeneous so loads never head-of-line block behind stores.
"""

import numpy as np

B, P, L, D = 16, 64, 256, 128
NCORES = 8
BPC = B // NCORES          # batches per core = 2
PAIRS = BPC * P            # (b,p) pairs per core = 128
GP = 16                    # pairs per group

ISQRT_D = 1.0 / np.sqrt(D)


def build_bass(pairs=PAIRS, gp=GP, sc_bufs=2, io_bufs=4, dve_pairs=4,
               head_split=4, tail_osplit=4, store_split=2, v_ring="gpsimd"):
    import concourse.bacc as bacc
    import concourse.mybir as mybir
    import concourse.tile as tile
    from concourse.bass import ds, ts

    f32 = mybir.dt.float32
    bf16 = mybir.dt.bfloat16
    groups = pairs // gp
    qw = gp * 256              # q/k row width (elements)
    vw = gp * 512              # v/out row width
    nc = bacc.Bacc("TRN2")

    # qt row (g*128+d): [j, c, p'] for pairs j in group g; l = 2p'+c
    qt = nc.dram_tensor("qt", [groups * 128, qw], bf16, kind="ExternalInput")
    # kt row (g*128+d): [j, m]
    kt = nc.dram_tensor("kt", [groups * 128, qw], bf16, kind="ExternalInput")
    # v row (g*128+p): [j, c, x] = V[pair j, l=2p+c, x]
    v = nc.dram_tensor("v", [groups * 128, vw], bf16, kind="ExternalInput")
    out = nc.dram_tensor("out", [groups * 128, vw], bf16, kind="ExternalOutput")

    mult = mybir.AluOpType.mult
    vload = {"gpsimd": nc.gpsimd, "sync": nc.sync, "vector": nc.vector}[v_ring]
    dw = dve_pairs * 512       # elements per DVE op
    nops = gp // dve_pairs

    with tile.TileContext(nc) as tc:
        with (
            tc.tile_pool(name="io", bufs=io_bufs) as io,
            tc.tile_pool(name="pss", bufs=sc_bufs, space="PSUM") as pss,
        ):
            for g in range(groups):
                r0 = g * 128
                qn = io.tile([128, qw], bf16, tag="qn")
                kn = io.tile([128, qw], bf16, tag="kn")
                vn = io.tile([128, vw], bf16, tag="vn")
                osb = io.tile([128, vw], bf16, tag="osb")

                nsplit = head_split if g == 0 else 1
                qh, vh = qw // nsplit, vw // nsplit
                for s in range(nsplit):
                    nc.sync.dma_start(
                        out=qn[:, ds(s * qh, qh)],
                        in_=qt[r0 : r0 + 128, s * qh : (s + 1) * qh],
                    )
                    nc.sync.dma_start(
                        out=kn[:, ds(s * qh, qh)],
                        in_=kt[r0 : r0 + 128, s * qh : (s + 1) * qh],
                    )
                    vload.dma_start(
                        out=vn[:, ds(s * vh, vh)],
                        in_=v[r0 : r0 + 128, s * vh : (s + 1) * vh],
                    )

                osplit = tail_osplit if g == groups - 1 else store_split
                for u in range(nops):
                    sc = pss.tile([128, dw], f32, tag="sc")
                    for q in range(dve_pairs):
                        j = dve_pairs * u + q
                        for r in range(2):
                            nc.tensor.matmul(
                                sc[:, ds(q * 512 + r * 256, 256)],
                                lhsT=qn[:, ds(j * 256 + r * 128, 128)],
                                rhs=kn[:, ds(j * 256, 256)],
                                start=True,
                                stop=True,
                            )
                    nc.vector.scalar_tensor_tensor(
                        out=osb[:, ds(u * dw, dw)],
                        in0=sc[:, ds(0, dw)],
                        scalar=1.0,
                        in1=vn[:, ds(u * dw, dw)],
                        op0=mult,
                        op1=mult,
                    )
                    per = nops // osplit
                    if (u + 1) % per == 0:
                        s0 = (u + 1 - per) * dw
                        nc.scalar.dma_start(
                            out=out[r0 : r0 + 128, s0 : s0 + per * dw],
                            in_=osb[:, ds(s0, per * dw)],
                        )
    nc.finalize()
    return nc


def make_in_maps(queries, keys, values, mask, ncores=NCORES, gp=GP):
    import ml_dtypes

    bf16 = ml_dtypes.bfloat16
    groups = PAIRS // gp
    queries = np.asarray(queries, dtype=np.float32)
    keys = np.asarray(keys, dtype=np.float32)
    values = np.asarray(values, dtype=np.float32)
    mask = np.asarray(mask, dtype=np.float32)
    in_maps = []
    for c in range(ncores):
        bs = slice(c * BPC, (c + 1) * BPC)
        mrow = mask[bs].reshape(PAIRS) * ISQRT_D
        qs = queries[bs].reshape(PAIRS, L, D) * mrow[:, None, None]
        ks = keys[bs].reshape(PAIRS, L, D)
        # qt: [g, j, p', c, d] -> [g, d, j, c, p'] ; l = 2p'+c
        qtp = (
            qs.reshape(groups, gp, 128, 2, D)
            .transpose(0, 4, 1, 3, 2)
            .reshape(groups * 128, gp * 256)
        )
        # kt: [g, j, m, d] -> [g, d, j, m]
        ktp = (
            ks.reshape(groups, gp, 256, D)
            .transpose(0, 3, 1, 2)
            .reshape(groups * 128, gp * 256)
        )
        # v: [g, j, p, c, x] -> [g, p, j, c, x] ; row l = 2p+c
        vp = (
            values[bs]
            .reshape(groups, gp, 128, 2, 256)
            .transpose(0, 2, 1, 3, 4)
            .reshape(groups * 128, gp * 512)
        )
        in_maps.append(
            {
                "qt": np.ascontiguousarray(qtp).astype(bf16),
                "kt": np.ascontiguousarray(ktp).astype(bf16),
                "v": np.ascontiguousarray(vp).astype(bf16),
            }
        )
    return in_maps


def unpack_out(arr, gp=GP):
    """[groups*128, gp*512] device layout -> [BPC, P, L, L] fp32."""
    groups = PAIRS // gp
    a = arr.astype(np.float32).reshape(groups, 128, gp, 2, 256)
    a = a.transpose(0, 2, 1, 3, 4).reshape(BPC, P, L, L)
    return a


def run(queries, keys, values, mask, trace=False, **build_kwargs):
    """Build, compile and run on 8 cores; returns (full_output, BassKernelResults)."""
    from concourse.bass_utils import run_bass_kernel_spmd

    gp = build_kwargs.get("gp", GP)
    nc = build_bass(**build_kwargs)
    in_maps = make_in_maps(queries, keys, values, mask, gp=gp)
    res = run_bass_kernel_spmd(
        nc, in_maps, core_ids=list(range(NCORES)), trace=trace
    )
    outs = [unpack_out(r["out"], gp=gp) for r in res.results]
    return np.concatenate(outs, axis=0), res


def kernel(queries, keys, values, mask):
    out, _ = run(queries, keys, values, mask, trace=False)
    return out


# revision 17
# speedup vs baseline: 1.2294x; 1.0583x over previous
"""Trainium2 Bass kernel for masked dot-product attention variant:

    out[b,p,l,m] = (sum_d Q[b,p,l,d] K[b,p,m,d]) / sqrt(D) * mask[b,p] * V[b,p,l,m]

Sharding: data-parallel over batch dim B=16 -> 2 batches per core on 8 cores.
Per core: 128 independent (b,p) pairs, each a 256x128 @ 128x256 gemm plus an
elementwise multiply with V (mask/sqrt(D) is folded into Q on the host).

All bulk I/O moves in bf16 (harness gate is rel_err < 2e-2; bf16 rounding
contributes ~5e-3), halving HBM traffic vs fp32. Layouts give 8KB (q/k) and
16KB (v/out) contiguous runs per DMA descriptor; with all 8 cores pulling,
the chip HBM wall (~2.6-2.8TB/s) caps each core around 330-400GB/s, so the
kernel is paced by its ~126us of DMA-engine busy time per core.

  qt/kt[g*128 + d, :] = 16 pairs packed per 128-row block (partition d)
  v/out[g*128 + p, :] = partition p holds rows l=2p, 2p+1 of each pair
                        (column-interleaved scores match this layout)

Per pair the PE computes scores[l_chunk, m] = qT[:, l_chunk].T @ kT in fp32
PSUM; one DVE scalar_tensor_tensor per FOUR pairs does out = scores * V with
bf16 output (batched to amortize DVE per-op overhead; ~72us DVE busy).
DMA rings: q/k loads on SP, v loads on GpSimd (SWDGE), stores on ACT - each
ring homogeneous so loads never head-of-line block behind stores.
"""

import numpy as np

B, P, L, D = 16, 64, 256, 128
NCORES = 8
BPC = B // NCORES          # batches per core = 2
PAIRS = BPC * P            # (b,p) pairs per core = 128
GP = 16                    # pairs per group

ISQRT_D = 1.0 / np.sqrt(D)


def build_bass(pairs=PAIRS, gp=GP, qkg=2 * GP, sc_bufs=2, qk_bufs=3, v_bufs=5,
               o_bufs=3, dve_pairs=4, head_split=4, store_pairs=8,
               v_ring="sync", store_ring="scalar", v_first=True, gp_tail_ops=0):
    """qkg: pairs per q/k DRAM row block (>= gp, multiple of gp). qkg=32
    gives 16KB q/k descriptor runs; gp=16 keeps v/out at 16KB runs.
    store_pairs: pairs per osb tile / store DMA (own pool of o_bufs tiles so
    DVE is decoupled from store drain at fine granularity)."""
    import concourse.bacc as bacc
    import concourse.mybir as mybir
    import concourse.tile as tile
    from concourse.bass import ds, ts

    f32 = mybir.dt.float32
    bf16 = mybir.dt.bfloat16
    groups = pairs // gp
    lgroups = pairs // qkg
    qw = qkg * 256             # q/k row width (elements)
    vw = gp * 512              # v/out row width
    nc = bacc.Bacc("TRN2")

    # qt row (lg*128+d): [j, c, p'] for pairs j in load-group lg; l = 2p'+c
    qt = nc.dram_tensor("qt", [lgroups * 128, qw], bf16, kind="ExternalInput")
    # kt row (lg*128+d): [j, m]
    kt = nc.dram_tensor("kt", [lgroups * 128, qw], bf16, kind="ExternalInput")
    # v row (g*128+p): [j, c, x] = V[pair j, l=2p+c, x]
    v = nc.dram_tensor("v", [groups * 128, vw], bf16, kind="ExternalInput")
    out = nc.dram_tensor("out", [groups * 128, vw], bf16, kind="ExternalOutput")

    mult = mybir.AluOpType.mult
    vload = {"gpsimd": nc.gpsimd, "sync": nc.sync, "scalar": nc.scalar}[v_ring]
    sstore = {"gpsimd": nc.gpsimd, "sync": nc.sync, "scalar": nc.scalar}[store_ring]
    dw = dve_pairs * 512       # elements per DVE op
    nops = gp // dve_pairs
    cg_per_lg = qkg // gp
    sw = store_pairs * 512     # elements per osb tile / store
    ops_per_store = store_pairs // dve_pairs

    with tile.TileContext(nc) as tc:
        with (
            tc.tile_pool(name="qk", bufs=qk_bufs) as qkp,
            tc.tile_pool(name="vp", bufs=v_bufs) as vp,
            tc.tile_pool(name="op", bufs=o_bufs) as op,
            tc.tile_pool(name="pss", bufs=sc_bufs, space="PSUM") as pss,
        ):
            qn = kn = None
            for g in range(groups):
                r0 = g * 128
                h = g % cg_per_lg
                if v_first:
                    vn = vp.tile([128, vw], bf16, tag="vn")
                    vsplit = head_split if g == 0 else 1
                    vh = vw // vsplit
                    for s in range(vsplit):
                        vload.dma_start(
                            out=vn[:, ds(s * vh, vh)],
                            in_=v[r0 : r0 + 128, s * vh : (s + 1) * vh],
                        )
                if h == 0:
                    lg = g // cg_per_lg
                    l0 = lg * 128
                    qn = qkp.tile([128, qw], bf16, tag="qn")
                    kn = qkp.tile([128, qw], bf16, tag="kn")
                    nsplit = head_split if lg == 0 else 1
                    qh = qw // nsplit
                    for s in range(nsplit):
                        nc.sync.dma_start(
                            out=qn[:, ds(s * qh, qh)],
                            in_=qt[l0 : l0 + 128, s * qh : (s + 1) * qh],
                        )
                        nc.sync.dma_start(
                            out=kn[:, ds(s * qh, qh)],
                            in_=kt[l0 : l0 + 128, s * qh : (s + 1) * qh],
                        )

                if not v_first:
                    vn = vp.tile([128, vw], bf16, tag="vn")
                    vsplit = head_split if g == 0 else 1
                    vh = vw // vsplit
                    for s in range(vsplit):
                        vload.dma_start(
                            out=vn[:, ds(s * vh, vh)],
                            in_=v[r0 : r0 + 128, s * vh : (s + 1) * vh],
                        )

                for u in range(nops):
                    if u % ops_per_store == 0:
                        osb = op.tile([128, sw], bf16, tag="osb")
                    sc = pss.tile([128, dw], f32, tag="sc")
                    for q in range(dve_pairs):
                        j = dve_pairs * u + q
                        jcol = (h * gp + j) * 256
                        for r in range(2):
                            nc.tensor.matmul(
                                sc[:, ds(q * 512 + r * 256, 256)],
                                lhsT=qn[:, ds(jcol + r * 128, 128)],
                                rhs=kn[:, ds(jcol, 256)],
                                start=True,
                                stop=True,
                            )
                    uo = u % ops_per_store
                    # optionally offload the last op(s) of each group to the
                    # otherwise-idle GpSimd engine to shorten the compute tail
                    eng = (
                        nc.gpsimd
                        if u >= nops - gp_tail_ops
                        else nc.vector
                    )
                    eng.scalar_tensor_tensor(
                        out=osb[:, ds(uo * dw, dw)],
                        in0=sc[:, ds(0, dw)],
                        scalar=1.0,
                        in1=vn[:, ds(u * dw, dw)],
                        op0=mult,
                        op1=mult,
                    )
                    if uo == ops_per_store - 1:
                        s0 = (u + 1 - ops_per_store) * dw
                        sstore.dma_start(
                            out=out[r0 : r0 + 128, s0 : s0 + sw],
                            in_=osb[:, ds(0, sw)],
                        )
    nc.finalize()
    return nc


def make_in_maps(queries, keys, values, mask, ncores=NCORES, gp=GP, qkg=GP):
    import ml_dtypes

    bf16 = ml_dtypes.bfloat16
    groups = PAIRS // gp
    lgroups = PAIRS // qkg
    queries = np.asarray(queries, dtype=np.float32)
    keys = np.asarray(keys, dtype=np.float32)
    values = np.asarray(values, dtype=np.float32)
    mask = np.asarray(mask, dtype=np.float32)
    in_maps = []
    for c in range(ncores):
        bs = slice(c * BPC, (c + 1) * BPC)
        mrow = mask[bs].reshape(PAIRS) * ISQRT_D
        qs = queries[bs].reshape(PAIRS, L, D) * mrow[:, None, None]
        ks = keys[bs].reshape(PAIRS, L, D)
        # qt: [lg, j, p', c, d] -> [lg, d, j, c, p'] ; l = 2p'+c
        qtp = (
            qs.reshape(lgroups, qkg, 128, 2, D)
            .transpose(0, 4, 1, 3, 2)
            .reshape(lgroups * 128, qkg * 256)
        )
        # kt: [lg, j, m, d] -> [lg, d, j, m]
        ktp = (
            ks.reshape(lgroups, qkg, 256, D)
            .transpose(0, 3, 1, 2)
            .reshape(lgroups * 128, qkg * 256)
        )
        # v: [g, j, p, c, x] -> [g, p, j, c, x] ; row l = 2p+c
        vp = (
            values[bs]
            .reshape(groups, gp, 128, 2, 256)
            .transpose(0, 2, 1, 3, 4)
            .reshape(groups * 128, gp * 512)
        )
        in_maps.append(
            {
                "qt": np.ascontiguousarray(qtp).astype(bf16),
                "kt": np.ascontiguousarray(ktp).astype(bf16),
                "v": np.ascontiguousarray(vp).astype(bf16),
            }
        )
    return in_maps


def unpack_out(arr, gp=GP):
    """[groups*128, gp*512] device layout -> [BPC, P, L, L] fp32."""
    groups = PAIRS // gp
    a = arr.astype(np.float32).reshape(groups, 128, gp, 2, 256)
    a = a.transpose(0, 2, 1, 3, 4).reshape(BPC, P, L, L)
    return a


def run(queries, keys, values, mask, trace=False, **build_kwargs):
    """Build, compile and run on 8 cores; returns (full_output, BassKernelResults)."""
    from concourse.bass_utils import run_bass_kernel_spmd

    gp = build_kwargs.get("gp", GP)
    qkg = build_kwargs.get("qkg", 2 * GP)
    nc = build_bass(**build_kwargs)
    in_maps = make_in_maps(queries, keys, values, mask, gp=gp, qkg=qkg)
    res = run_bass_kernel_spmd(
        nc, in_maps, core_ids=list(range(NCORES)), trace=trace
    )
    outs = [unpack_out(r["out"], gp=gp) for r in res.results]
    return np.concatenate(outs, axis=0), res


def kernel(queries, keys, values, mask):
    out, _ = run(queries, keys, values, mask, trace=False)
    return out
